# revision 16
# baseline (speedup 1.0000x reference)
"""Causal single-head attention on 8 NeuronCores (Trainium2, Bass/Tile).

Problem: x[16,4096,128] fp32; Wq/Wk/Wv[128,128]; y = softmax(mask(QK^T/sqrt(128))) @ V.
Sharding: data-parallel over batch, 2 batches per core, no collectives.

Per-batch on-core plan (P^T-stationary PV so the output lands in natural
[q, h] orientation AND the softmax denominators fall out of the same
matmuls -- no separate ones@P^T pass, no O^T transposes):
  xT = PE-transpose(x)            [d, L]   fp32 -> fp16
  qT = Wq^T @ xT, kT = Wk^T @ xT  [h, L]   fp16
  Vaug = [x @ Wv | ones]          [l, 129] fp16 per 128-l tile (stride 132)
  per 512-wide q-block J, k-tiles paired into [128, <=1024] S^T strips:
    S^T = kT_i^T @ qT_J           (PSUM fp32)
    diag tiles: S^T[:, :128] += causal_mask (-1e30 strict lower)
    P^T = exp(scale * S^T)        one wide ACT per pair, PSUM->SBUF, fp16
    per k-tile i, q-subtile g:    oacc_g[q, 0:129] += P_i^T[:,g]^T @ Vaug_i
      (col 128 of oacc_g = sum_k P = softmax denominator)
  epilogue per g: rcp = 1/oacc[:,128]; y = oacc[:, :128] * rcp; direct
  natural-layout [128,128] store (contiguous 64KB in DRAM).

oacc PSUM tiles hold two 129-col groups per bank; only the bank's
chronologically-first matmul uses start=True (start clears has_written
for the WHOLE bank), later groups' first matmuls rely on flags=0
overwrite-where-unwritten semantics.

x loads move 2KB contiguous per partition per descriptor; the 4x row
interleave this creates is undone inside the PE transposes.
"""
import sys

if '/opt/trn_rl_repo' not in sys.path:
    sys.path.insert(0, '/opt/trn_rl_repo')

import numpy as np

B, L, D, H = 16, 4096, 128, 128
NCORES = 8
BPC = B // NCORES          # batches per core
QB = 512                   # q block width
NQB = L // QB              # 8 q blocks
KT = 128                   # k tile width
NKT = L // KT              # 32 k tiles
CHUNK = 512                # phase-A l-chunk
NCHUNK = L // CHUNK        # 8
VST = 132                  # Vaug col stride (129 used: 128 V + ones)
SCALE = float(1.0 / np.sqrt(H))
NEG = -1.0e30

_cache = {}


def _build():
    import concourse.mybir as mybir
    import concourse.tile as tile
    from concourse import bacc

    f32 = mybir.dt.float32
    f16 = mybir.dt.float16
    i16 = mybir.dt.int16
    EXP = mybir.ActivationFunctionType.Exp
    CPY = mybir.ActivationFunctionType.Copy
    MUL = mybir.AluOpType.mult
    ADD = mybir.AluOpType.add
    # Schraudolph exp for fp16: bits = round(S * EA + EB); bitcast -> fp16
    # approximates exp(SCALE*S) within ~4% (DVE round-to-nearest + saturate;
    # the -1e30 causal mask saturates to -32768 = fp16 -0.0).
    EA = float(1024.0 * np.log2(np.e) * SCALE)
    EB = 15298.0

    nc = bacc.Bacc("TRN2", target_bir_lowering=False, debug=False,
                   num_devices=NCORES)
    x_ap = nc.dram_tensor("x", [BPC, L, D], f32, kind="ExternalInput").ap()
    wq_ap = nc.dram_tensor("Wq", [D, H], f32, kind="ExternalInput").ap()
    wk_ap = nc.dram_tensor("Wk", [D, H], f32, kind="ExternalInput").ap()
    wv_ap = nc.dram_tensor("Wv", [D, H], f32, kind="ExternalInput").ap()
    id_ap = nc.dram_tensor("ident", [128, 128], f32, kind="ExternalInput").ap()
    mk_ap = nc.dram_tensor("mask", [128, 128], f32, kind="ExternalInput").ap()
    y_ap = nc.dram_tensor("y", [BPC, L, H], f32, kind="ExternalOutput").ap()

    with tile.TileContext(nc) as tc:
        with (
            tc.tile_pool(name="const", bufs=1) as constp,
            tc.tile_pool(name="xchunk", bufs=3) as xchp,
            tc.tile_pool(name="xt", bufs=3) as xtp,
            tc.tile_pool(name="qkv", bufs=BPC) as qkvp,
            tc.tile_pool(name="pt", bufs=4) as ptp,
            tc.tile_pool(name="rcp", bufs=4) as rcpp,
            tc.tile_pool(name="ysb", bufs=4) as yp,
            tc.tile_pool(name="ps_wide", bufs=2, space="PSUM") as ps_wide,
            tc.tile_pool(name="ps_acc", bufs=4, space="PSUM") as ps_acc,
        ):
            # ---- constants ----
            ident = constp.tile([128, 128], f32, tag="ident")
            nc.sync.dma_start(ident[:], id_ap[:])
            mask = constp.tile([128, 128], f32, tag="mask")
            nc.sync.dma_start(mask[:], mk_ap[:])
            # 0/1 mask in fp16 for the post-exp multiply: exp(0)=1 keeps,
            # exp(-1e30)=0 kills (computed once on ACT from the additive mask)
            mask01 = constp.tile([128, 128], f16, tag="mask01")
            nc.scalar.activation(mask01[:], mask[:], EXP)
            w_h = {}
            for name, ap in (("q", wq_ap), ("k", wk_ap), ("v", wv_ap)):
                wf = constp.tile([128, 128], f32, tag=f"w{name}f")
                nc.sync.dma_start(wf[:], ap[:])
                wh = constp.tile([128, 128], f16, tag=f"w{name}h")
                nc.vector.tensor_copy(wh[:], wf[:])
                w_h[name] = wh
            ones_f = constp.tile([128, 32], f32, tag="ones_f")
            nc.gpsimd.memset(ones_f[:], 1.0)
            ones_h = constp.tile([128, 32], f16, tag="ones_h")
            nc.vector.tensor_copy(ones_h[:], ones_f[:])
            zeros_f = constp.tile([128, 128], f32, tag="zeros_f")
            nc.gpsimd.memset(zeros_f[:], 0.0)

            # ---- per-batch tensors ----
            qT = {}
            kT = {}
            Vn = {}
            for b in range(BPC):
                qT[b] = qkvp.tile([128, L], f16, tag="qT", name=f"qT{b}")
                kT[b] = qkvp.tile([128, L], f16, tag="kT", name=f"kT{b}")
                Vn[b] = qkvp.tile([128, NKT * VST], f16, tag="V", name=f"V{b}")

            # ---- phase A: transpose + projections ----
            # x chunk DMA loads 2KB contiguous per partition: partition p gets
            # rows {512c+4p+a: a=0..3}. The a-th [128,128] sub-tile holds rows
            # l=512c+4p+a; its PE transpose yields columns l=4p+a, which the
            # DVE copy un-interleaves with a stride-4 write so xT ends up in
            # natural l order.
            xv = {}
            for b in range(BPC):
                xv[b] = x_ap[b].rearrange("(c p a) d -> c p (a d)", p=128, a=4)
                # ones column of every Vaug tile (V copies leave it intact)
                nc.vector.tensor_copy(Vn[b][:, 128:NKT * VST:VST], ones_h[:])
            # chunk-major, batch-inner: twice the independent work in flight
            # during the cold start (first chunk DMAs)
            for c in range(NCHUNK):
                for b in range(BPC):
                    xch = xchp.tile([128, 512], f32, tag="xch")
                    nc.sync.dma_start(xch[:], xv[b][c])
                    xt = xtp.tile([128, CHUNK], f16, tag="xt")
                    for n in range(4):
                        tp = ps_acc.tile([128, 128], f32, tag="acc")
                        nc.tensor.transpose(tp[:], xch[:, 128 * n:128 * (n + 1)],
                                            ident[:])
                        # un-interleave copies split across ACT and DVE
                        if n < 2:
                            nc.scalar.activation(xt[:, n:CHUNK:4], tp[:], CPY)
                        else:
                            nc.vector.tensor_copy(xt[:, n:CHUNK:4], tp[:])
                    # q^T, k^T chunks: [h, CHUNK]; copies on ACT (idle here)
                    for name, dst in (("q", qT[b]), ("k", kT[b])):
                        pp = ps_wide.tile([128, CHUNK], f32, tag="wide")
                        nc.tensor.matmul(pp[:], w_h[name][:], xt[:],
                                         start=True, stop=True)
                        nc.scalar.activation(
                            dst[:, CHUNK * c:CHUNK * (c + 1)], pp[:], CPY)
                    # V tiles: [l,h] per 128-l sub-tile, at VST col stride
                    for n in range(4):
                        vp = ps_acc.tile([128, 128], f32, tag="acc")
                        nc.tensor.matmul(vp[:], xt[:, 128 * n:128 * (n + 1)],
                                         w_h["v"][:], start=True, stop=True)
                        i = 4 * c + n
                        nc.vector.tensor_copy(
                            Vn[b][:, VST * i:VST * i + 128], vp[:])

            # ---- phase B: attention ----
            # PV matmuls run one pair BEHIND the S^T/exp stream so each
            # pair's exp latency hides under the previous pair's PE work.
            yv = {b: y_ap[b].rearrange("(J g p) h -> J p g h", p=128, g=4)
                  for b in range(BPC)}
            oacc = {}

            def emit_pv(p):
                b, J = p["b"], p["J"]
                if (b, J) not in oacc:
                    # oA holds q-subtiles g=0,1; oB holds g=2,3 (129 cols
                    # each: 128 output features + the exp-sum in col 128).
                    oacc[(b, J)] = (
                        ps_acc.tile([128, 258], f32, tag="acc",
                                    padded_shape=[128, 512],
                                    name=f"oA{b}_{J}"),
                        ps_acc.tile([128, 258], f32, tag="acc",
                                    padded_shape=[128, 512],
                                    name=f"oB{b}_{J}"),
                    )
                oA, oB = oacc[(b, J)]
                oT = {0: oA, 1: oA, 2: oB, 3: oB}
                ptw = p["ptw"]
                for i, qoff, off, N in p["entries"]:
                    di = i - 4 * J
                    for g in range(max(0, di), 4):
                        c0 = off + 128 * g - qoff
                        ot = oT[g]
                        col = 129 * (g % 2)
                        first = (i == 0 and g % 2 == 0)
                        last = (g % 2 == 1 and i == 4 * J + g)
                        nc.tensor.matmul(
                            ot[:, col:col + 129],
                            ptw[:, c0:c0 + 128],
                            Vn[b][:, VST * i:VST * i + 129],
                            start=first, stop=last,
                            skip_group_check=True)
                if p["last"]:
                    # epilogue: one PSUM->SBUF copy per accumulator (frees
                    # the PSUM bank fast), then cheap SBUF-side math
                    otc = yp.tile([128, 516], f32, tag="oc")
                    nc.vector.tensor_copy(otc[:, 0:258], oA[:])
                    nc.vector.tensor_copy(otc[:, 258:516], oB[:])
                    ys = yp.tile([128, 512], f32, tag="y")
                    for g in range(4):
                        col = 258 * (g // 2) + 129 * (g % 2)
                        rcp = rcpp.tile([128, 1], f32, tag="rcp")
                        nc.vector.reciprocal(rcp[:], otc[:, col + 128:col + 129])
                        # scalar_tensor_tensor is the fast fused path; the
                        # slow AP-scalar tensor_scalar ran ~650ns/op here
                        nc.vector.scalar_tensor_tensor(
                            ys[:, 128 * g:128 * (g + 1)],
                            otc[:, col:col + 128], rcp[:], zeros_f[:],
                            MUL, ADD)
                    nc.sync.dma_start(yv[b][J], ys[:])

            pend = []
            for b in range(BPC):
                for J in range(NQB):
                    # k-tiles paired into wide S^T strips: full pairs first,
                    # then the 4 diagonal tiles as two packed pairs.
                    pairs = []
                    for g in range(2 * J):
                        pairs.append((2 * g, 2 * g + 1))
                    pairs.append((4 * J, 4 * J + 1))
                    pairs.append((4 * J + 2, 4 * J + 3))

                    for idx, pair in enumerate(pairs):
                        stw = ps_wide.tile([128, 2 * QB], f32, tag="wide")
                        entries = []
                        cur = 0
                        for i in pair:
                            qoff = max(0, 128 * (i - 4 * J))
                            N = QB - qoff
                            nc.tensor.matmul(
                                stw[:, cur:cur + N],
                                kT[b][:, KT * i:KT * (i + 1)],
                                qT[b][:, QB * J + qoff:QB * (J + 1)],
                                start=True, stop=True,
                                skip_group_check=True)
                            entries.append((i, qoff, cur, N))
                            cur += N
                        ptw = ptp.tile([128, 2 * QB], f16, tag="pt")
                        # offload half of the full-width exps to the DVE via
                        # the Schraudolph bit-trick; diagonal pairs (largest
                        # attention weights) keep the exact ScalarE exp.
                        if cur == 2 * QB and idx % 2 == 1:
                            nc.vector.tensor_scalar(
                                ptw[:, :cur].bitcast(i16), stw[:, :cur],
                                EA, EB, MUL, ADD)
                        else:
                            nc.scalar.activation(ptw[:, :cur], stw[:, :cur],
                                                 EXP, scale=SCALE)
                        # causal mask applied post-exp (0/1 multiply) so the
                        # DVE never sits in the S^T -> exp critical chain;
                        # the 2-pair PV delay gives it plenty of slack.
                        for i, qoff, off, N in entries:
                            if i >= 4 * J:
                                nc.vector.tensor_mul(
                                    ptw[:, off:off + 128],
                                    ptw[:, off:off + 128], mask01[:])
                        if len(pend) == 2:
                            emit_pv(pend.pop(0))
                        pend.append({"b": b, "J": J, "entries": entries,
                                     "ptw": ptw,
                                     "last": idx == len(pairs) - 1})
            for p in pend:
                emit_pv(p)
    nc.compile()
    return nc


def _host_consts():
    ident = np.eye(128, dtype=np.float32)
    kk = np.arange(128)[:, None]
    qq = np.arange(128)[None, :]
    mask = np.where(qq >= kk, 0.0, NEG).astype(np.float32)
    return ident, mask


def kernel(x, Wq, Wk, Wv):
    from concourse import bass_utils

    if "nc" not in _cache:
        _cache["nc"] = _build()
    nc = _cache["nc"]

    x = np.ascontiguousarray(x, dtype=np.float32)
    ident, mask = _host_consts()
    in_maps = []
    for c in range(NCORES):
        in_maps.append({
            "x": x[BPC * c:BPC * (c + 1)],
            "Wq": np.ascontiguousarray(Wq, dtype=np.float32),
            "Wk": np.ascontiguousarray(Wk, dtype=np.float32),
            "Wv": np.ascontiguousarray(Wv, dtype=np.float32),
            "ident": ident,
            "mask": mask,
        })
    res = bass_utils.run_bass_kernel_spmd(nc, in_maps,
                                          core_ids=list(range(NCORES)))
    _cache["last_results"] = res
    y = np.concatenate([res.results[c]["y"] for c in range(NCORES)], axis=0)
    return y


# revision 19
# speedup vs baseline: 1.0106x; 1.0106x over previous
"""Causal single-head attention on 8 NeuronCores (Trainium2, Bass/Tile).

Problem: x[16,4096,128] fp32; Wq/Wk/Wv[128,128]; y = softmax(mask(QK^T/sqrt(128))) @ V.
Sharding: data-parallel over batch, 2 batches per core, no collectives.

Per-batch on-core plan (P^T-stationary PV so the output lands in natural
[q, h] orientation AND the softmax denominators fall out of the same
matmuls -- no separate ones@P^T pass, no O^T transposes):
  xT = PE-transpose(x)            [d, L]   fp32 -> fp16
  qT = Wq^T @ xT, kT = Wk^T @ xT  [h, L]   fp16
  Vaug = [x @ Wv | ones]          [l, 129] fp16 per 128-l tile (stride 132)
  per 512-wide q-block J, k-tiles paired into [128, <=1024] S^T strips:
    S^T = kT_i^T @ qT_J           (PSUM fp32)
    diag tiles: S^T[:, :128] += causal_mask (-1e30 strict lower)
    P^T = exp(scale * S^T)        one wide ACT per pair, PSUM->SBUF, fp16
    per k-tile i, q-subtile g:    oacc_g[q, 0:129] += P_i^T[:,g]^T @ Vaug_i
      (col 128 of oacc_g = sum_k P = softmax denominator)
  epilogue per g: rcp = 1/oacc[:,128]; y = oacc[:, :128] * rcp; direct
  natural-layout [128,128] store (contiguous 64KB in DRAM).

oacc PSUM tiles hold two 129-col groups per bank; only the bank's
chronologically-first matmul uses start=True (start clears has_written
for the WHOLE bank), later groups' first matmuls rely on flags=0
overwrite-where-unwritten semantics.

x loads move 2KB contiguous per partition per descriptor; the 4x row
interleave this creates is undone inside the PE transposes.
"""
import sys

if '/opt/trn_rl_repo' not in sys.path:
    sys.path.insert(0, '/opt/trn_rl_repo')

import numpy as np

B, L, D, H = 16, 4096, 128, 128
NCORES = 8
BPC = B // NCORES          # batches per core
QB = 512                   # q block width
NQB = L // QB              # 8 q blocks
KT = 128                   # k tile width
NKT = L // KT              # 32 k tiles
CHUNK = 512                # phase-A l-chunk
NCHUNK = L // CHUNK        # 8
VST = 132                  # Vaug col stride (129 used: 128 V + ones)
SCALE = float(1.0 / np.sqrt(H))
NEG = -1.0e30

_cache = {}


def _build():
    import concourse.mybir as mybir
    import concourse.tile as tile
    from concourse import bacc

    f32 = mybir.dt.float32
    f16 = mybir.dt.float16
    i16 = mybir.dt.int16
    EXP = mybir.ActivationFunctionType.Exp
    CPY = mybir.ActivationFunctionType.Copy
    MUL = mybir.AluOpType.mult
    ADD = mybir.AluOpType.add
    # Schraudolph exp for fp16: bits = round(S * EA + EB); bitcast -> fp16
    # approximates exp(SCALE*S) within ~4% (DVE round-to-nearest + saturate;
    # the -1e30 causal mask saturates to -32768 = fp16 -0.0).
    EA = float(1024.0 * np.log2(np.e) * SCALE)
    EB = 15298.0

    nc = bacc.Bacc("TRN2", target_bir_lowering=False, debug=False,
                   num_devices=NCORES)
    x_ap = nc.dram_tensor("x", [BPC, L, D], f32, kind="ExternalInput").ap()
    wq_ap = nc.dram_tensor("Wq", [D, H], f32, kind="ExternalInput").ap()
    wk_ap = nc.dram_tensor("Wk", [D, H], f32, kind="ExternalInput").ap()
    wv_ap = nc.dram_tensor("Wv", [D, H], f32, kind="ExternalInput").ap()
    id_ap = nc.dram_tensor("ident", [128, 128], f32, kind="ExternalInput").ap()
    mk_ap = nc.dram_tensor("mask", [128, 128], f32, kind="ExternalInput").ap()
    y_ap = nc.dram_tensor("y", [BPC, L, H], f32, kind="ExternalOutput").ap()

    with tile.TileContext(nc) as tc:
        with (
            tc.tile_pool(name="const", bufs=1) as constp,
            tc.tile_pool(name="xchunk", bufs=3) as xchp,
            tc.tile_pool(name="xt", bufs=3) as xtp,
            tc.tile_pool(name="qkv", bufs=BPC) as qkvp,
            tc.tile_pool(name="pt", bufs=4) as ptp,
            tc.tile_pool(name="rcp", bufs=4) as rcpp,
            tc.tile_pool(name="ysb", bufs=4) as yp,
            tc.tile_pool(name="ps_wide", bufs=2, space="PSUM") as ps_wide,
            tc.tile_pool(name="ps_acc", bufs=4, space="PSUM") as ps_acc,
        ):
            # ---- first x chunks in flight before anything else ----
            xv = {b: x_ap[b].rearrange("(c p a) d -> c p (a d)", p=128, a=4)
                  for b in range(BPC)}
            xch0 = {}
            for b in range(BPC):
                xch0[b] = xchp.tile([128, 512], f32, tag="xch",
                                    name=f"xch0_{b}")
                nc.sync.dma_start(xch0[b][:], xv[b][0])

            # ---- constants ----
            ident = constp.tile([128, 128], f32, tag="ident")
            nc.sync.dma_start(ident[:], id_ap[:])
            mask = constp.tile([128, 128], f32, tag="mask")
            nc.sync.dma_start(mask[:], mk_ap[:])
            # 0/1 mask in fp16 for the post-exp multiply: exp(0)=1 keeps,
            # exp(-1e30)=0 kills (computed once on ACT from the additive mask)
            mask01 = constp.tile([128, 128], f16, tag="mask01")
            nc.scalar.activation(mask01[:], mask[:], EXP)
            w_h = {}
            for name, ap in (("q", wq_ap), ("k", wk_ap), ("v", wv_ap)):
                wf = constp.tile([128, 128], f32, tag=f"w{name}f")
                nc.sync.dma_start(wf[:], ap[:])
                wh = constp.tile([128, 128], f16, tag=f"w{name}h")
                nc.vector.tensor_copy(wh[:], wf[:])
                w_h[name] = wh
            ones_f = constp.tile([128, 32], f32, tag="ones_f")
            nc.gpsimd.memset(ones_f[:], 1.0)
            ones_h = constp.tile([128, 32], f16, tag="ones_h")
            nc.vector.tensor_copy(ones_h[:], ones_f[:])
            zeros_f = constp.tile([128, 128], f32, tag="zeros_f")
            nc.gpsimd.memset(zeros_f[:], 0.0)

            # ---- per-batch tensors ----
            qT = {}
            kT = {}
            Vn = {}
            for b in range(BPC):
                qT[b] = qkvp.tile([128, L], f16, tag="qT", name=f"qT{b}")
                kT[b] = qkvp.tile([128, L], f16, tag="kT", name=f"kT{b}")
                Vn[b] = qkvp.tile([128, NKT * VST], f16, tag="V", name=f"V{b}")

            # ---- phase A: transpose + projections ----
            # x chunk DMA loads 2KB contiguous per partition: partition p gets
            # rows {512c+4p+a: a=0..3}. The a-th [128,128] sub-tile holds rows
            # l=512c+4p+a; its PE transpose yields columns l=4p+a, which the
            # DVE copy un-interleaves with a stride-4 write so xT ends up in
            # natural l order.
            for b in range(BPC):
                # ones column of every Vaug tile (V copies leave it intact)
                nc.vector.tensor_copy(Vn[b][:, 128:NKT * VST:VST], ones_h[:])
            # chunk-major, batch-inner: twice the independent work in flight
            # during the cold start (first chunk DMAs)
            for c in range(NCHUNK):
                for b in range(BPC):
                    if c == 0:
                        xch = xch0[b]
                    else:
                        xch = xchp.tile([128, 512], f32, tag="xch")
                        nc.sync.dma_start(xch[:], xv[b][c])
                    xt = xtp.tile([128, CHUNK], f16, tag="xt")
                    for n in range(4):
                        tp = ps_acc.tile([128, 128], f32, tag="acc")
                        nc.tensor.transpose(tp[:], xch[:, 128 * n:128 * (n + 1)],
                                            ident[:])
                        # un-interleave copies split across ACT and DVE
                        if n < 2:
                            nc.scalar.activation(xt[:, n:CHUNK:4], tp[:], CPY)
                        else:
                            nc.vector.tensor_copy(xt[:, n:CHUNK:4], tp[:])
                    # q^T, k^T chunks: [h, CHUNK]; copies on ACT (idle here)
                    for name, dst in (("q", qT[b]), ("k", kT[b])):
                        pp = ps_wide.tile([128, CHUNK], f32, tag="wide")
                        nc.tensor.matmul(pp[:], w_h[name][:], xt[:],
                                         start=True, stop=True)
                        nc.scalar.activation(
                            dst[:, CHUNK * c:CHUNK * (c + 1)], pp[:], CPY)
                    # V tiles: [l,h] per 128-l sub-tile, at VST col stride
                    for n in range(4):
                        vp = ps_acc.tile([128, 128], f32, tag="acc")
                        nc.tensor.matmul(vp[:], xt[:, 128 * n:128 * (n + 1)],
                                         w_h["v"][:], start=True, stop=True)
                        i = 4 * c + n
                        nc.vector.tensor_copy(
                            Vn[b][:, VST * i:VST * i + 128], vp[:])

            # ---- phase B: attention ----
            # PV matmuls run one pair BEHIND the S^T/exp stream so each
            # pair's exp latency hides under the previous pair's PE work.
            yv = {b: y_ap[b].rearrange("(J g p) h -> J p g h", p=128, g=4)
                  for b in range(BPC)}
            oacc = {}

            def emit_pv(p):
                b, J = p["b"], p["J"]
                if (b, J) not in oacc:
                    # oA holds q-subtiles g=0,1; oB holds g=2,3 (129 cols
                    # each: 128 output features + the exp-sum in col 128).
                    oacc[(b, J)] = (
                        ps_acc.tile([128, 258], f32, tag="acc",
                                    padded_shape=[128, 512],
                                    name=f"oA{b}_{J}"),
                        ps_acc.tile([128, 258], f32, tag="acc",
                                    padded_shape=[128, 512],
                                    name=f"oB{b}_{J}"),
                    )
                oA, oB = oacc[(b, J)]
                oT = {0: oA, 1: oA, 2: oB, 3: oB}
                ptw = p["ptw"]
                for i, qoff, off, N in p["entries"]:
                    di = i - 4 * J
                    for g in range(max(0, di), 4):
                        c0 = off + 128 * g - qoff
                        ot = oT[g]
                        col = 129 * (g % 2)
                        first = (i == 0 and g % 2 == 0)
                        last = (g % 2 == 1 and i == 4 * J + g)
                        nc.tensor.matmul(
                            ot[:, col:col + 129],
                            ptw[:, c0:c0 + 128],
                            Vn[b][:, VST * i:VST * i + 129],
                            start=first, stop=last,
                            skip_group_check=True)
                if p["last"]:
                    # epilogue: one PSUM->SBUF copy per accumulator (frees
                    # the PSUM bank fast), then cheap SBUF-side math
                    otc = yp.tile([128, 516], f32, tag="oc")
                    nc.vector.tensor_copy(otc[:, 0:258], oA[:])
                    nc.vector.tensor_copy(otc[:, 258:516], oB[:])
                    ys = yp.tile([128, 512], f32, tag="y")
                    for g in range(4):
                        col = 258 * (g // 2) + 129 * (g % 2)
                        rcp = rcpp.tile([128, 1], f32, tag="rcp")
                        nc.vector.reciprocal(rcp[:], otc[:, col + 128:col + 129])
                        # scalar_tensor_tensor is the fast fused path; the
                        # slow AP-scalar tensor_scalar ran ~650ns/op here
                        nc.vector.scalar_tensor_tensor(
                            ys[:, 128 * g:128 * (g + 1)],
                            otc[:, col:col + 128], rcp[:], zeros_f[:],
                            MUL, ADD)
                    nc.sync.dma_start(yv[b][J], ys[:])

            pend = []
            for b in range(BPC):
                for J in range(NQB):
                    # k-tiles paired into wide S^T strips: full pairs first,
                    # then the 4 diagonal tiles as two packed pairs.
                    pairs = []
                    for g in range(2 * J):
                        pairs.append((2 * g, 2 * g + 1))
                    pairs.append((4 * J, 4 * J + 1))
                    pairs.append((4 * J + 2, 4 * J + 3))

                    for idx, pair in enumerate(pairs):
                        stw = ps_wide.tile([128, 2 * QB], f32, tag="wide")
                        entries = []
                        cur = 0
                        for i in pair:
                            qoff = max(0, 128 * (i - 4 * J))
                            N = QB - qoff
                            nc.tensor.matmul(
                                stw[:, cur:cur + N],
                                kT[b][:, KT * i:KT * (i + 1)],
                                qT[b][:, QB * J + qoff:QB * (J + 1)],
                                start=True, stop=True,
                                skip_group_check=True)
                            entries.append((i, qoff, cur, N))
                            cur += N
                        ptw = ptp.tile([128, 2 * QB], f16, tag="pt")
                        # offload ~1/3 of the full-width exps to the DVE via
                        # the Schraudolph bit-trick (runs 1x from PSUM, so it
                        # only load-balances ACT vs DVE); diagonal pairs keep
                        # the exact ScalarE exp.
                        if cur == 2 * QB and idx % 3 == 1:
                            nc.vector.tensor_scalar(
                                ptw[:, :cur].bitcast(i16), stw[:, :cur],
                                EA, EB, MUL, ADD)
                        else:
                            nc.scalar.activation(ptw[:, :cur], stw[:, :cur],
                                                 EXP, scale=SCALE)
                        # causal mask applied post-exp (0/1 multiply) so the
                        # DVE never sits in the S^T -> exp critical chain;
                        # the 2-pair PV delay gives it plenty of slack.
                        for i, qoff, off, N in entries:
                            if i >= 4 * J:
                                nc.vector.tensor_mul(
                                    ptw[:, off:off + 128],
                                    ptw[:, off:off + 128], mask01[:])
                        if len(pend) == 2:
                            emit_pv(pend.pop(0))
                        pend.append({"b": b, "J": J, "entries": entries,
                                     "ptw": ptw,
                                     "last": idx == len(pairs) - 1})
            for p in pend:
                emit_pv(p)
    nc.compile()
    return nc


def _host_consts():
    ident = np.eye(128, dtype=np.float32)
    kk = np.arange(128)[:, None]
    qq = np.arange(128)[None, :]
    mask = np.where(qq >= kk, 0.0, NEG).astype(np.float32)
    return ident, mask


def kernel(x, Wq, Wk, Wv):
    from concourse import bass_utils

    if "nc" not in _cache:
        _cache["nc"] = _build()
    nc = _cache["nc"]

    x = np.ascontiguousarray(x, dtype=np.float32)
    ident, mask = _host_consts()
    in_maps = []
    for c in range(NCORES):
        in_maps.append({
            "x": x[BPC * c:BPC * (c + 1)],
            "Wq": np.ascontiguousarray(Wq, dtype=np.float32),
            "Wk": np.ascontiguousarray(Wk, dtype=np.float32),
            "Wv": np.ascontiguousarray(Wv, dtype=np.float32),
            "ident": ident,
            "mask": mask,
        })
    res = bass_utils.run_bass_kernel_spmd(nc, in_maps,
                                          core_ids=list(range(NCORES)))
    _cache["last_results"] = res
    y = np.concatenate([res.results[c]["y"] for c in range(NCORES)], axis=0)
    return y


# revision 20
# speedup vs baseline: 1.0175x; 1.0068x over previous
"""Causal single-head attention on 8 NeuronCores (Trainium2, Bass/Tile).

Problem: x[16,4096,128] fp32; Wq/Wk/Wv[128,128]; y = softmax(mask(QK^T/sqrt(128))) @ V.
Sharding: data-parallel over batch, 2 batches per core, no collectives.

Per-batch on-core plan (P^T-stationary PV so the output lands in natural
[q, h] orientation AND the softmax denominators fall out of the same
matmuls -- no separate ones@P^T pass, no O^T transposes):
  xT = PE-transpose(x)            [d, L]   fp32 -> fp16
  qT = Wq^T @ xT, kT = Wk^T @ xT  [h, L]   fp16
  Vaug = [x @ Wv | ones]          [l, 129] fp16 per 128-l tile (stride 132)
  per 512-wide q-block J, k-tiles paired into [128, <=1024] S^T strips:
    S^T = kT_i^T @ qT_J           (PSUM fp32)
    diag tiles: S^T[:, :128] += causal_mask (-1e30 strict lower)
    P^T = exp(scale * S^T)        one wide ACT per pair, PSUM->SBUF, fp16
    per k-tile i, q-subtile g:    oacc_g[q, 0:129] += P_i^T[:,g]^T @ Vaug_i
      (col 128 of oacc_g = sum_k P = softmax denominator)
  epilogue per g: rcp = 1/oacc[:,128]; y = oacc[:, :128] * rcp; direct
  natural-layout [128,128] store (contiguous 64KB in DRAM).

oacc PSUM tiles hold two 129-col groups per bank; only the bank's
chronologically-first matmul uses start=True (start clears has_written
for the WHOLE bank), later groups' first matmuls rely on flags=0
overwrite-where-unwritten semantics.

x loads move 2KB contiguous per partition per descriptor; the 4x row
interleave this creates is undone inside the PE transposes.
"""
import sys

if '/opt/trn_rl_repo' not in sys.path:
    sys.path.insert(0, '/opt/trn_rl_repo')

import numpy as np

B, L, D, H = 16, 4096, 128, 128
NCORES = 8
BPC = B // NCORES          # batches per core
QB = 512                   # q block width
NQB = L // QB              # 8 q blocks
KT = 128                   # k tile width
NKT = L // KT              # 32 k tiles
CHUNK = 512                # phase-A l-chunk
NCHUNK = L // CHUNK        # 8
VST = 132                  # Vaug col stride (129 used: 128 V + ones)
SCALE = float(1.0 / np.sqrt(H))
NEG = -1.0e30

_cache = {}


def _build():
    import concourse.mybir as mybir
    import concourse.tile as tile
    from concourse import bacc

    f32 = mybir.dt.float32
    f16 = mybir.dt.float16
    i16 = mybir.dt.int16
    EXP = mybir.ActivationFunctionType.Exp
    CPY = mybir.ActivationFunctionType.Copy
    MUL = mybir.AluOpType.mult
    ADD = mybir.AluOpType.add
    # Schraudolph exp for fp16: bits = round(S * EA + EB); bitcast -> fp16
    # approximates exp(SCALE*S) within ~4% (DVE round-to-nearest + saturate;
    # the -1e30 causal mask saturates to -32768 = fp16 -0.0).
    EA = float(1024.0 * np.log2(np.e) * SCALE)
    EB = 15298.0

    nc = bacc.Bacc("TRN2", target_bir_lowering=False, debug=False,
                   num_devices=NCORES)
    x_ap = nc.dram_tensor("x", [BPC, L, D], f32, kind="ExternalInput").ap()
    wq_ap = nc.dram_tensor("Wq", [D, H], f32, kind="ExternalInput").ap()
    wk_ap = nc.dram_tensor("Wk", [D, H], f32, kind="ExternalInput").ap()
    wv_ap = nc.dram_tensor("Wv", [D, H], f32, kind="ExternalInput").ap()
    id_ap = nc.dram_tensor("ident", [128, 128], f32, kind="ExternalInput").ap()
    mk_ap = nc.dram_tensor("mask", [128, 128], f32, kind="ExternalInput").ap()
    y_ap = nc.dram_tensor("y", [BPC, L, H], f32, kind="ExternalOutput").ap()

    with tile.TileContext(nc) as tc:
        with (
            tc.tile_pool(name="const", bufs=1) as constp,
            tc.tile_pool(name="xchunk", bufs=3) as xchp,
            tc.tile_pool(name="xt", bufs=3) as xtp,
            tc.tile_pool(name="qkv", bufs=BPC) as qkvp,
            tc.tile_pool(name="pt", bufs=4) as ptp,
            tc.tile_pool(name="rcp", bufs=4) as rcpp,
            tc.tile_pool(name="ysb", bufs=4) as yp,
            tc.tile_pool(name="ps_wide", bufs=2, space="PSUM") as ps_wide,
            tc.tile_pool(name="ps_acc", bufs=4, space="PSUM") as ps_acc,
        ):
            # ---- first x chunks in flight before anything else ----
            xv = {b: x_ap[b].rearrange("(c p a) d -> c p (a d)", p=128, a=4)
                  for b in range(BPC)}
            xch0 = {}
            for b in range(BPC):
                xch0[b] = xchp.tile([128, 512], f32, tag="xch",
                                    name=f"xch0_{b}")
                nc.sync.dma_start(xch0[b][:], xv[b][0])

            # ---- constants ----
            ident = constp.tile([128, 128], f32, tag="ident")
            nc.sync.dma_start(ident[:], id_ap[:])
            mask = constp.tile([128, 128], f32, tag="mask")
            nc.sync.dma_start(mask[:], mk_ap[:])
            # 0/1 mask in fp16 for the post-exp multiply: exp(0)=1 keeps,
            # exp(-1e30)=0 kills (computed once on ACT from the additive mask)
            mask01 = constp.tile([128, 128], f16, tag="mask01")
            nc.scalar.activation(mask01[:], mask[:], EXP)
            w_h = {}
            for name, ap in (("q", wq_ap), ("k", wk_ap), ("v", wv_ap)):
                wf = constp.tile([128, 128], f32, tag=f"w{name}f")
                nc.sync.dma_start(wf[:], ap[:])
                wh = constp.tile([128, 128], f16, tag=f"w{name}h")
                nc.vector.tensor_copy(wh[:], wf[:])
                w_h[name] = wh
            ones_f = constp.tile([128, 32], f32, tag="ones_f")
            nc.gpsimd.memset(ones_f[:], 1.0)
            ones_h = constp.tile([128, 32], f16, tag="ones_h")
            nc.vector.tensor_copy(ones_h[:], ones_f[:])
            zeros_f = constp.tile([128, 128], f32, tag="zeros_f")
            nc.gpsimd.memset(zeros_f[:], 0.0)

            # ---- per-batch tensors ----
            qT = {}
            kT = {}
            Vn = {}
            for b in range(BPC):
                qT[b] = qkvp.tile([128, L], f16, tag="qT", name=f"qT{b}")
                kT[b] = qkvp.tile([128, L], f16, tag="kT", name=f"kT{b}")
                Vn[b] = qkvp.tile([128, NKT * VST], f16, tag="V", name=f"V{b}")

            # ---- phase A: transpose + projections ----
            # x chunk DMA loads 2KB contiguous per partition: partition p gets
            # rows {512c+4p+a: a=0..3}. The a-th [128,128] sub-tile holds rows
            # l=512c+4p+a; its PE transpose yields columns l=4p+a, which the
            # DVE copy un-interleaves with a stride-4 write so xT ends up in
            # natural l order.
            for b in range(BPC):
                # ones column of every Vaug tile (V copies leave it intact)
                nc.vector.tensor_copy(Vn[b][:, 128:NKT * VST:VST], ones_h[:])
            # chunk-major, batch-inner: twice the independent work in flight
            # during the cold start (first chunk DMAs)
            for c in range(NCHUNK):
                for b in range(BPC):
                    if c == 0:
                        xch = xch0[b]
                    else:
                        xch = xchp.tile([128, 512], f32, tag="xch")
                        nc.sync.dma_start(xch[:], xv[b][c])
                    xt = xtp.tile([128, CHUNK], f16, tag="xt")
                    for n in range(4):
                        tp = ps_acc.tile([128, 128], f32, tag="acc")
                        nc.tensor.transpose(tp[:], xch[:, 128 * n:128 * (n + 1)],
                                            ident[:])
                        # un-interleave copies split across ACT and DVE
                        if n < 2:
                            nc.scalar.activation(xt[:, n:CHUNK:4], tp[:], CPY)
                        else:
                            nc.vector.tensor_copy(xt[:, n:CHUNK:4], tp[:])
                    # q^T, k^T chunks: [h, CHUNK]; copies on ACT (idle here)
                    for name, dst in (("q", qT[b]), ("k", kT[b])):
                        pp = ps_wide.tile([128, CHUNK], f32, tag="wide")
                        nc.tensor.matmul(pp[:], w_h[name][:], xt[:],
                                         start=True, stop=True)
                        nc.scalar.activation(
                            dst[:, CHUNK * c:CHUNK * (c + 1)], pp[:], CPY)
                    # V tiles: [l,h] per 128-l sub-tile, at VST col stride
                    for n in range(4):
                        vp = ps_acc.tile([128, 128], f32, tag="acc")
                        nc.tensor.matmul(vp[:], xt[:, 128 * n:128 * (n + 1)],
                                         w_h["v"][:], start=True, stop=True)
                        i = 4 * c + n
                        nc.vector.tensor_copy(
                            Vn[b][:, VST * i:VST * i + 128], vp[:])

            # ---- phase B: attention ----
            # PV matmuls run one pair BEHIND the S^T/exp stream so each
            # pair's exp latency hides under the previous pair's PE work.
            yv = {b: y_ap[b].rearrange("(J g p) h -> J p g h", p=128, g=4)
                  for b in range(BPC)}
            oacc = {}

            def emit_pv(p):
                b, J = p["b"], p["J"]
                if (b, J) not in oacc:
                    # oA holds q-subtiles g=0,1; oB holds g=2,3 (129 cols
                    # each: 128 output features + the exp-sum in col 128).
                    oacc[(b, J)] = (
                        ps_acc.tile([128, 258], f32, tag="acc",
                                    padded_shape=[128, 512],
                                    name=f"oA{b}_{J}"),
                        ps_acc.tile([128, 258], f32, tag="acc",
                                    padded_shape=[128, 512],
                                    name=f"oB{b}_{J}"),
                    )
                oA, oB = oacc[(b, J)]
                oT = {0: oA, 1: oA, 2: oB, 3: oB}
                ptw = p["ptw"]
                for i, qoff, off, N in p["entries"]:
                    di = i - 4 * J
                    for g in range(max(0, di), 4):
                        c0 = off + 128 * g - qoff
                        ot = oT[g]
                        col = 129 * (g % 2)
                        first = (i == 0 and g % 2 == 0)
                        last = (g % 2 == 1 and i == 4 * J + g)
                        nc.tensor.matmul(
                            ot[:, col:col + 129],
                            ptw[:, c0:c0 + 128],
                            Vn[b][:, VST * i:VST * i + 129],
                            start=first, stop=last,
                            skip_group_check=True)
                if p["last"]:
                    # epilogue: one PSUM->SBUF copy per accumulator (frees
                    # the PSUM bank fast), then cheap SBUF-side math
                    otc = yp.tile([128, 516], f32, tag="oc")
                    nc.vector.tensor_copy(otc[:, 0:258], oA[:])
                    nc.vector.tensor_copy(otc[:, 258:516], oB[:])
                    ys = yp.tile([128, 512], f32, tag="y")
                    for g in range(4):
                        col = 258 * (g // 2) + 129 * (g % 2)
                        rcp = rcpp.tile([128, 1], f32, tag="rcp")
                        nc.vector.reciprocal(rcp[:], otc[:, col + 128:col + 129])
                        # scalar_tensor_tensor is the fast fused path; the
                        # slow AP-scalar tensor_scalar ran ~650ns/op here
                        nc.vector.scalar_tensor_tensor(
                            ys[:, 128 * g:128 * (g + 1)],
                            otc[:, col:col + 128], rcp[:], zeros_f[:],
                            MUL, ADD)
                    nc.sync.dma_start(yv[b][J], ys[:])

            pend = []
            # J-outer, batch-inner: at every J boundary the other batch's
            # pair stream provides independent PE work to hide epilogue and
            # exp latency behind.
            for J in range(NQB):
                for b in range(BPC):
                    # k-tiles paired into wide S^T strips: full pairs first,
                    # then the 4 diagonal tiles as two packed pairs.
                    pairs = []
                    for g in range(2 * J):
                        pairs.append((2 * g, 2 * g + 1))
                    pairs.append((4 * J, 4 * J + 1))
                    pairs.append((4 * J + 2, 4 * J + 3))

                    for idx, pair in enumerate(pairs):
                        stw = ps_wide.tile([128, 2 * QB], f32, tag="wide")
                        entries = []
                        cur = 0
                        for i in pair:
                            qoff = max(0, 128 * (i - 4 * J))
                            N = QB - qoff
                            nc.tensor.matmul(
                                stw[:, cur:cur + N],
                                kT[b][:, KT * i:KT * (i + 1)],
                                qT[b][:, QB * J + qoff:QB * (J + 1)],
                                start=True, stop=True,
                                skip_group_check=True)
                            entries.append((i, qoff, cur, N))
                            cur += N
                        ptw = ptp.tile([128, 2 * QB], f16, tag="pt")
                        # offload ~1/3 of the full-width exps to the DVE via
                        # the Schraudolph bit-trick (runs 1x from PSUM, so it
                        # only load-balances ACT vs DVE); diagonal pairs keep
                        # the exact ScalarE exp.
                        if cur == 2 * QB and idx % 3 == 1:
                            nc.vector.tensor_scalar(
                                ptw[:, :cur].bitcast(i16), stw[:, :cur],
                                EA, EB, MUL, ADD)
                        else:
                            nc.scalar.activation(ptw[:, :cur], stw[:, :cur],
                                                 EXP, scale=SCALE)
                        # causal mask applied post-exp (0/1 multiply) so the
                        # DVE never sits in the S^T -> exp critical chain;
                        # the 2-pair PV delay gives it plenty of slack.
                        for i, qoff, off, N in entries:
                            if i >= 4 * J:
                                nc.vector.tensor_mul(
                                    ptw[:, off:off + 128],
                                    ptw[:, off:off + 128], mask01[:])
                        if len(pend) == 2:
                            emit_pv(pend.pop(0))
                        pend.append({"b": b, "J": J, "entries": entries,
                                     "ptw": ptw,
                                     "last": idx == len(pairs) - 1})
            for p in pend:
                emit_pv(p)
    nc.compile()
    return nc


def _host_consts():
    ident = np.eye(128, dtype=np.float32)
    kk = np.arange(128)[:, None]
    qq = np.arange(128)[None, :]
    mask = np.where(qq >= kk, 0.0, NEG).astype(np.float32)
    return ident, mask


def kernel(x, Wq, Wk, Wv):
    from concourse import bass_utils

    if "nc" not in _cache:
        _cache["nc"] = _build()
    nc = _cache["nc"]

    x = np.ascontiguousarray(x, dtype=np.float32)
    ident, mask = _host_consts()
    in_maps = []
    for c in range(NCORES):
        in_maps.append({
            "x": x[BPC * c:BPC * (c + 1)],
            "Wq": np.ascontiguousarray(Wq, dtype=np.float32),
            "Wk": np.ascontiguousarray(Wk, dtype=np.float32),
            "Wv": np.ascontiguousarray(Wv, dtype=np.float32),
            "ident": ident,
            "mask": mask,
        })
    res = bass_utils.run_bass_kernel_spmd(nc, in_maps,
                                          core_ids=list(range(NCORES)))
    _cache["last_results"] = res
    y = np.concatenate([res.results[c]["y"] for c in range(NCORES)], axis=0)
    return y


# revision 21
# speedup vs baseline: 1.0263x; 1.0086x over previous
"""Causal single-head attention on 8 NeuronCores (Trainium2, Bass/Tile).

Problem: x[16,4096,128] fp32; Wq/Wk/Wv[128,128]; y = softmax(mask(QK^T/sqrt(128))) @ V.
Sharding: data-parallel over batch, 2 batches per core, no collectives.

Per-batch on-core plan (P^T-stationary PV so the output lands in natural
[q, h] orientation AND the softmax denominators fall out of the same
matmuls -- no separate ones@P^T pass, no O^T transposes):
  xT = PE-transpose(x)            [d, L]   fp32 -> fp16
  qT = Wq^T @ xT, kT = Wk^T @ xT  [h, L]   fp16
  Vaug = [x @ Wv | ones]          [l, 129] fp16 per 128-l tile (stride 132)
  per 512-wide q-block J, k-tiles paired into [128, <=1024] S^T strips:
    S^T = kT_i^T @ qT_J           (PSUM fp32)
    diag tiles: S^T[:, :128] += causal_mask (-1e30 strict lower)
    P^T = exp(scale * S^T)        one wide ACT per pair, PSUM->SBUF, fp16
    per k-tile i, q-subtile g:    oacc_g[q, 0:129] += P_i^T[:,g]^T @ Vaug_i
      (col 128 of oacc_g = sum_k P = softmax denominator)
  epilogue per g: rcp = 1/oacc[:,128]; y = oacc[:, :128] * rcp; direct
  natural-layout [128,128] store (contiguous 64KB in DRAM).

oacc PSUM tiles hold two 129-col groups per bank; only the bank's
chronologically-first matmul uses start=True (start clears has_written
for the WHOLE bank), later groups' first matmuls rely on flags=0
overwrite-where-unwritten semantics.

x loads move 2KB contiguous per partition per descriptor; the 4x row
interleave this creates is undone inside the PE transposes.
"""
import sys

if '/opt/trn_rl_repo' not in sys.path:
    sys.path.insert(0, '/opt/trn_rl_repo')

import numpy as np

B, L, D, H = 16, 4096, 128, 128
NCORES = 8
BPC = B // NCORES          # batches per core
QB = 512                   # q block width
NQB = L // QB              # 8 q blocks
KT = 128                   # k tile width
NKT = L // KT              # 32 k tiles
CHUNK = 512                # phase-A l-chunk
NCHUNK = L // CHUNK        # 8
VST = 132                  # Vaug col stride (129 used: 128 V + ones)
SCALE = float(1.0 / np.sqrt(H))
NEG = -1.0e30

_cache = {}


def _build():
    import concourse.mybir as mybir
    import concourse.tile as tile
    from concourse import bacc

    f32 = mybir.dt.float32
    f16 = mybir.dt.float16
    i16 = mybir.dt.int16
    EXP = mybir.ActivationFunctionType.Exp
    CPY = mybir.ActivationFunctionType.Copy
    MUL = mybir.AluOpType.mult
    ADD = mybir.AluOpType.add
    # Schraudolph exp for fp16: bits = round(S * EA + EB); bitcast -> fp16
    # approximates exp(SCALE*S) within ~4% (DVE round-to-nearest + saturate;
    # the -1e30 causal mask saturates to -32768 = fp16 -0.0).
    EA = float(1024.0 * np.log2(np.e) * SCALE)
    EB = 15298.0

    nc = bacc.Bacc("TRN2", target_bir_lowering=False, debug=False,
                   num_devices=NCORES)
    x_ap = nc.dram_tensor("x", [BPC, L, D], f32, kind="ExternalInput").ap()
    wq_ap = nc.dram_tensor("Wq", [D, H], f32, kind="ExternalInput").ap()
    wk_ap = nc.dram_tensor("Wk", [D, H], f32, kind="ExternalInput").ap()
    wv_ap = nc.dram_tensor("Wv", [D, H], f32, kind="ExternalInput").ap()
    id_ap = nc.dram_tensor("ident", [128, 128], f32, kind="ExternalInput").ap()
    mk_ap = nc.dram_tensor("mask", [128, 128], f32, kind="ExternalInput").ap()
    y_ap = nc.dram_tensor("y", [BPC, L, H], f32, kind="ExternalOutput").ap()

    with tile.TileContext(nc) as tc:
        with (
            tc.tile_pool(name="const", bufs=1) as constp,
            tc.tile_pool(name="xchunk", bufs=3) as xchp,
            tc.tile_pool(name="xt", bufs=3) as xtp,
            tc.tile_pool(name="qkv", bufs=BPC) as qkvp,
            tc.tile_pool(name="pt", bufs=5) as ptp,
            tc.tile_pool(name="rcp", bufs=4) as rcpp,
            tc.tile_pool(name="ysb", bufs=4) as yp,
            tc.tile_pool(name="ps_wide", bufs=2, space="PSUM") as ps_wide,
            tc.tile_pool(name="ps_acc", bufs=4, space="PSUM") as ps_acc,
        ):
            # ---- first x chunks in flight before anything else ----
            xv = {b: x_ap[b].rearrange("(c p a) d -> c p (a d)", p=128, a=4)
                  for b in range(BPC)}
            xch0 = {}
            for b in range(BPC):
                xch0[b] = xchp.tile([128, 512], f32, tag="xch",
                                    name=f"xch0_{b}")
                nc.sync.dma_start(xch0[b][:], xv[b][0])

            # ---- constants ----
            ident = constp.tile([128, 128], f32, tag="ident")
            nc.sync.dma_start(ident[:], id_ap[:])
            mask = constp.tile([128, 128], f32, tag="mask")
            nc.sync.dma_start(mask[:], mk_ap[:])
            # 0/1 mask in fp16 for the post-exp multiply: exp(0)=1 keeps,
            # exp(-1e30)=0 kills (computed once on ACT from the additive mask)
            mask01 = constp.tile([128, 128], f16, tag="mask01")
            nc.scalar.activation(mask01[:], mask[:], EXP)
            w_h = {}
            for name, ap in (("q", wq_ap), ("k", wk_ap), ("v", wv_ap)):
                wf = constp.tile([128, 128], f32, tag=f"w{name}f")
                nc.sync.dma_start(wf[:], ap[:])
                wh = constp.tile([128, 128], f16, tag=f"w{name}h")
                nc.vector.tensor_copy(wh[:], wf[:])
                w_h[name] = wh
            ones_f = constp.tile([128, 32], f32, tag="ones_f")
            nc.gpsimd.memset(ones_f[:], 1.0)
            ones_h = constp.tile([128, 32], f16, tag="ones_h")
            nc.vector.tensor_copy(ones_h[:], ones_f[:])
            zeros_f = constp.tile([128, 128], f32, tag="zeros_f")
            nc.gpsimd.memset(zeros_f[:], 0.0)

            # ---- per-batch tensors ----
            qT = {}
            kT = {}
            Vn = {}
            for b in range(BPC):
                qT[b] = qkvp.tile([128, L], f16, tag="qT", name=f"qT{b}")
                kT[b] = qkvp.tile([128, L], f16, tag="kT", name=f"kT{b}")
                Vn[b] = qkvp.tile([128, NKT * VST], f16, tag="V", name=f"V{b}")

            # ---- phase A: transpose + projections ----
            # x chunk DMA loads 2KB contiguous per partition: partition p gets
            # rows {512c+4p+a: a=0..3}. The a-th [128,128] sub-tile holds rows
            # l=512c+4p+a; its PE transpose yields columns l=4p+a, which the
            # DVE copy un-interleaves with a stride-4 write so xT ends up in
            # natural l order.
            for b in range(BPC):
                # ones column of every Vaug tile (V copies leave it intact)
                nc.vector.tensor_copy(Vn[b][:, 128:NKT * VST:VST], ones_h[:])
            # chunk-major, batch-inner: twice the independent work in flight
            # during the cold start (first chunk DMAs)
            for c in range(NCHUNK):
                for b in range(BPC):
                    if c == 0:
                        xch = xch0[b]
                    else:
                        xch = xchp.tile([128, 512], f32, tag="xch")
                        nc.sync.dma_start(xch[:], xv[b][c])
                    xt = xtp.tile([128, CHUNK], f16, tag="xt")
                    for n in range(4):
                        tp = ps_acc.tile([128, 128], f32, tag="acc")
                        nc.tensor.transpose(tp[:], xch[:, 128 * n:128 * (n + 1)],
                                            ident[:])
                        # un-interleave copies split across ACT and DVE
                        if n < 2:
                            nc.scalar.activation(xt[:, n:CHUNK:4], tp[:], CPY)
                        else:
                            nc.vector.tensor_copy(xt[:, n:CHUNK:4], tp[:])
                    # q^T, k^T chunks: [h, CHUNK]; copies on ACT (idle here)
                    for name, dst in (("q", qT[b]), ("k", kT[b])):
                        pp = ps_wide.tile([128, CHUNK], f32, tag="wide")
                        nc.tensor.matmul(pp[:], w_h[name][:], xt[:],
                                         start=True, stop=True)
                        nc.scalar.activation(
                            dst[:, CHUNK * c:CHUNK * (c + 1)], pp[:], CPY)
                    # V tiles: [l,h] per 128-l sub-tile, at VST col stride
                    for n in range(4):
                        vp = ps_acc.tile([128, 128], f32, tag="acc")
                        nc.tensor.matmul(vp[:], xt[:, 128 * n:128 * (n + 1)],
                                         w_h["v"][:], start=True, stop=True)
                        i = 4 * c + n
                        nc.vector.tensor_copy(
                            Vn[b][:, VST * i:VST * i + 128], vp[:])

            # ---- phase B: attention ----
            # PV matmuls run one pair BEHIND the S^T/exp stream so each
            # pair's exp latency hides under the previous pair's PE work.
            yv = {b: y_ap[b].rearrange("(J g p) h -> J p g h", p=128, g=4)
                  for b in range(BPC)}
            oacc = {}

            def emit_pv(p):
                b, J = p["b"], p["J"]
                if (b, J) not in oacc:
                    # oA holds q-subtiles g=0,1; oB holds g=2,3 (129 cols
                    # each: 128 output features + the exp-sum in col 128).
                    oacc[(b, J)] = (
                        ps_acc.tile([128, 258], f32, tag="acc",
                                    padded_shape=[128, 512],
                                    name=f"oA{b}_{J}"),
                        ps_acc.tile([128, 258], f32, tag="acc",
                                    padded_shape=[128, 512],
                                    name=f"oB{b}_{J}"),
                    )
                oA, oB = oacc[(b, J)]
                oT = {0: oA, 1: oA, 2: oB, 3: oB}
                ptw = p["ptw"]
                for i, qoff, off, N in p["entries"]:
                    di = i - 4 * J
                    for g in range(max(0, di), 4):
                        c0 = off + 128 * g - qoff
                        ot = oT[g]
                        col = 129 * (g % 2)
                        first = (i == 0 and g % 2 == 0)
                        last = (g % 2 == 1 and i == 4 * J + g)
                        nc.tensor.matmul(
                            ot[:, col:col + 129],
                            ptw[:, c0:c0 + 128],
                            Vn[b][:, VST * i:VST * i + 129],
                            start=first, stop=last,
                            skip_group_check=True)
                if p["last"]:
                    # epilogue: one PSUM->SBUF copy per accumulator (frees
                    # the PSUM bank fast), then cheap SBUF-side math
                    otc = yp.tile([128, 516], f32, tag="oc")
                    nc.vector.tensor_copy(otc[:, 0:258], oA[:])
                    nc.vector.tensor_copy(otc[:, 258:516], oB[:])
                    ys = yp.tile([128, 512], f32, tag="y")
                    for g in range(4):
                        col = 258 * (g // 2) + 129 * (g % 2)
                        rcp = rcpp.tile([128, 1], f32, tag="rcp")
                        nc.vector.reciprocal(rcp[:], otc[:, col + 128:col + 129])
                        # scalar_tensor_tensor is the fast fused path; the
                        # slow AP-scalar tensor_scalar ran ~650ns/op here
                        nc.vector.scalar_tensor_tensor(
                            ys[:, 128 * g:128 * (g + 1)],
                            otc[:, col:col + 128], rcp[:], zeros_f[:],
                            MUL, ADD)
                    nc.sync.dma_start(yv[b][J], ys[:])

            pend = []
            # J-outer, batch-inner: at every J boundary the other batch's
            # pair stream provides independent PE work to hide epilogue and
            # exp latency behind.
            for J in range(NQB):
                for b in range(BPC):
                    # k-tiles paired into wide S^T strips: full pairs first,
                    # then the 4 diagonal tiles as two packed pairs.
                    pairs = []
                    for g in range(2 * J):
                        pairs.append((2 * g, 2 * g + 1))
                    pairs.append((4 * J, 4 * J + 1))
                    pairs.append((4 * J + 2, 4 * J + 3))

                    for idx, pair in enumerate(pairs):
                        stw = ps_wide.tile([128, 2 * QB], f32, tag="wide")
                        entries = []
                        cur = 0
                        for i in pair:
                            qoff = max(0, 128 * (i - 4 * J))
                            N = QB - qoff
                            nc.tensor.matmul(
                                stw[:, cur:cur + N],
                                kT[b][:, KT * i:KT * (i + 1)],
                                qT[b][:, QB * J + qoff:QB * (J + 1)],
                                start=True, stop=True,
                                skip_group_check=True)
                            entries.append((i, qoff, cur, N))
                            cur += N
                        ptw = ptp.tile([128, 2 * QB], f16, tag="pt")
                        # offload ~1/3 of the full-width exps to the DVE via
                        # the Schraudolph bit-trick (runs 1x from PSUM, so it
                        # only load-balances ACT vs DVE); diagonal pairs keep
                        # the exact ScalarE exp.
                        if cur == 2 * QB and idx % 5 in (1, 3):
                            nc.vector.tensor_scalar(
                                ptw[:, :cur].bitcast(i16), stw[:, :cur],
                                EA, EB, MUL, ADD)
                        else:
                            nc.scalar.activation(ptw[:, :cur], stw[:, :cur],
                                                 EXP, scale=SCALE)
                        # causal mask applied post-exp (0/1 multiply) so the
                        # DVE never sits in the S^T -> exp critical chain;
                        # the 2-pair PV delay gives it plenty of slack.
                        for i, qoff, off, N in entries:
                            if i >= 4 * J:
                                nc.vector.tensor_mul(
                                    ptw[:, off:off + 128],
                                    ptw[:, off:off + 128], mask01[:])
                        if len(pend) == 3:
                            emit_pv(pend.pop(0))
                        pend.append({"b": b, "J": J, "entries": entries,
                                     "ptw": ptw,
                                     "last": idx == len(pairs) - 1})
            for p in pend:
                emit_pv(p)
    nc.compile()
    return nc


def _host_consts():
    ident = np.eye(128, dtype=np.float32)
    kk = np.arange(128)[:, None]
    qq = np.arange(128)[None, :]
    mask = np.where(qq >= kk, 0.0, NEG).astype(np.float32)
    return ident, mask


def kernel(x, Wq, Wk, Wv):
    from concourse import bass_utils

    if "nc" not in _cache:
        _cache["nc"] = _build()
    nc = _cache["nc"]

    x = np.ascontiguousarray(x, dtype=np.float32)
    ident, mask = _host_consts()
    in_maps = []
    for c in range(NCORES):
        in_maps.append({
            "x": x[BPC * c:BPC * (c + 1)],
            "Wq": np.ascontiguousarray(Wq, dtype=np.float32),
            "Wk": np.ascontiguousarray(Wk, dtype=np.float32),
            "Wv": np.ascontiguousarray(Wv, dtype=np.float32),
            "ident": ident,
            "mask": mask,
        })
    res = bass_utils.run_bass_kernel_spmd(nc, in_maps,
                                          core_ids=list(range(NCORES)))
    _cache["last_results"] = res
    y = np.concatenate([res.results[c]["y"] for c in range(NCORES)], axis=0)
    return y
